# revision 19
# baseline (speedup 1.0000x reference)
"""MoE layer (16 experts, top-2) on 8 Trainium2 NeuronCores, expert-parallel.

Strategy:
  - Host computes the gating (logits -> top-k -> softmax) and routes tokens
    into per-expert buckets (the shard/dispatch step).
  - Experts are sorted by token count; the 8 largest go to slot 0 (padded to
    C0 = max count), the 8 smallest to slot 1 (padded to C1 = 9th largest
    count).  One big + one small expert per core: per-core padded work is
    C0 + C1 ~ 1080 tokens instead of 2*C0.
  - All matmuls in bf16 (full PE rate + FWL fast weight loads + half the HBM
    traffic of fp32), fp32 PSUM accumulation.
  - matmul1: ht[f, t] = silu(W1.T @ xt + b1), f on partitions, tokens moving.
  - matmul2: y[d, t]  = W2.T @ ht, d on partitions, tokens moving -- weights
    are always the stationary operand and the moving dim is the exact token
    count (no ceil(C/128) partition-padding waste).
  - Host combines: out[token] = sum over its top-k experts of
    gate * y[:, token] (the unshard/combine step; gate applied on host).
"""

import math

import numpy as np

D_MODEL = 1024
D_FF = 4096
N_EXPERTS = 16
N_CORES = 8
SLOTS = 2  # experts per core
KD = D_MODEL // 128  # 8 contraction chunks for matmul1 / output chunks for mm2
KF = D_FF // 128  # 32 f chunks

_PROG_CACHE: dict[tuple, object] = {}


def _split_tokens(c):
    """Split token count c into moving-dim tiles <= 512 (PSUM bank limit),
    as equal as possible (each >= 256 for c >= 512)."""
    n = max(1, math.ceil(c / 512))
    q, r = divmod(c, n)
    sizes = [q + (1 if i < r else 0) for i in range(n)]
    out = []
    c0 = 0
    for sz in sizes:
        out.append((c0, sz))
        c0 += sz
    return out


def _build_program(C0, C1):
    import concourse.bass as bass  # noqa: F401
    import concourse.tile as tile
    from concourse import bacc, mybir

    f32 = mybir.dt.float32
    bf16 = mybir.dt.bfloat16
    silu = mybir.ActivationFunctionType.Silu

    nc = bacc.Bacc("TRN2", target_bir_lowering=False, debug=False, num_devices=N_CORES)

    CS = [C0, C1]
    xt_d, w1_d, w2_d, b1_d, y_d = [], [], [], [], []
    for s, C in enumerate(CS):
        xt_d.append(nc.dram_tensor(f"xt{s}", [128, KD, C], bf16, kind="ExternalInput").ap())
        w1_d.append(nc.dram_tensor(f"w1_{s}", [KF, 128, KD, 128], bf16, kind="ExternalInput").ap())
        w2_d.append(nc.dram_tensor(f"w2_{s}", [KD, 128, KF, 128], bf16, kind="ExternalInput").ap())
        b1_d.append(nc.dram_tensor(f"b1_{s}", [128, KF], f32, kind="ExternalInput").ap())
        y_d.append(nc.dram_tensor(f"y{s}", [KD, 128, C], f32, kind="ExternalOutput").ap())

    with tile.TileContext(nc) as tc:
        with (
            tc.tile_pool(name="xtp", bufs=1) as xtp,
            tc.tile_pool(name="w1p", bufs=12) as w1p,
            tc.tile_pool(name="w2p", bufs=3) as w2p,
            tc.tile_pool(name="htp", bufs=1) as htp,
            tc.tile_pool(name="smallp", bufs=2) as smallp,
            tc.tile_pool(name="yp", bufs=4) as yp,
            tc.tile_pool(name="ps1", bufs=4, space="PSUM") as ps1,
            tc.tile_pool(name="ps2", bufs=4, space="PSUM") as ps2,
        ):
            # (No HAM pre-warm: the startup window is HBM-bound — a warm PE
            # just outruns the x/W1 supply and stalls; measured net-negative.)
            # Small slot first: its x block is smaller, so the PE starts
            # slightly earlier.
            sorder = [1, 0]
            for oi, s in enumerate(sorder):
                C = CS[s]
                tiles = _split_tokens(C)

                # ---- loads for this expert ----
                # per-kd x chunks on two queues so the first matmul can start
                # as soon as chunk 0 + the first W1 block land
                # chunks in consumption order (kd descending), spread over all
                # three queues, ahead of the w1 stream on sync
                xt = xtp.tile([128, KD, C], bf16, name=f"xt{s}", tag="xt")
                nc.sync.dma_start(xt[:, 6:8], xt_d[s][:, 6:8])
                nc.scalar.dma_start(xt[:, 4:6], xt_d[s][:, 4:6])
                nc.gpsimd.dma_start(xt[:, 2:4], xt_d[s][:, 2:4])
                nc.scalar.dma_start(xt[:, 0:2], xt_d[s][:, 0:2])
                b1t = smallp.tile([128, KF], f32, name=f"b1t{s}", tag="b1t")
                nc.gpsimd.dma_start(b1t[:], b1_d[s])

                # ---- matmul1: ht[f, c] = silu(W1.T @ xt + b1) ----
                # w2 blocks: the first two ride the sync queue positioned
                # behind 8 w1 issues (keeps them out of the startup window);
                # the rest are WAR-gated just-in-time by the bufs=2 rotation
                # against matmul2's progress.
                ht = htp.tile([128, KF, C], bf16, name=f"ht{s}", tag="ht")
                w2ts = []
                for kf in range(KF):
                    w1t = w1p.tile([128, KD, 128], bf16, name=f"w1t{s}_{kf}", tag="w1t")
                    nc.sync.dma_start(w1t[:], w1_d[s][kf])
                    if kf in (8, 12, 14) or (16 <= kf and kf % 2 == 0 and len(w2ts) < KD):
                        kd = len(w2ts)
                        w2t = w2p.tile(
                            [128, KF, 128], bf16, name=f"w2t{s}_{kd}", tag="w2t"
                        )
                        # first three ride the scalar DMA ring (idle, and
                        # queue-ordered behind this slot's earlier silus);
                        # the rest are WAR-gated just-in-time on gpsimd
                        w2eng = nc.scalar if kd < 3 else nc.gpsimd
                        w2eng.dma_start(w2t[:], w2_d[s][kd])
                        w2ts.append(w2t)
                    pt = [
                        ps1.tile([128, 512], f32, name=f"ps1_{s}_{kf}_{i}", tag="ps1")
                        for i in range(len(tiles))
                    ]
                    # kd descending: the first matmul gates on the last x
                    # chunk, so the w1 stream banks a cushion during the
                    # x load instead of stuttering chunk-by-chunk
                    for j, kd in enumerate(reversed(range(KD))):
                        for p, (c0, tw) in zip(pt, tiles):
                            nc.tensor.matmul(
                                p[:, :tw],
                                lhsT=w1t[:, kd],
                                rhs=xt[:, kd, c0 : c0 + tw],
                                start=(j == 0),
                                stop=(j == KD - 1),
                            )
                    for p, (c0, tw) in zip(pt, tiles):
                        nc.scalar.activation(
                            ht[:, kf, c0 : c0 + tw],
                            p[:, :tw],
                            silu,
                            bias=b1t[:, kf : kf + 1],
                        )

                # ---- matmul2: y[d, c] = W2.T @ ht ----
                for kd in range(KD):
                    w2t = w2ts[kd]
                    pt2 = [
                        ps2.tile([128, 512], f32, name=f"ps2_{s}_{kd}_{i}", tag="ps2")
                        for i in range(len(tiles))
                    ]
                    for kf in range(KF):
                        for p, (c0, tw) in zip(pt2, tiles):
                            nc.tensor.matmul(
                                p[:, :tw],
                                lhsT=w2t[:, kf],
                                rhs=ht[:, kf, c0 : c0 + tw],
                                start=(kf == 0),
                                stop=(kf == KF - 1),
                            )
                    last_group = oi == len(sorder) - 1 and kd == KD - 1
                    for i, (p, (c0, tw)) in enumerate(zip(pt2, tiles)):
                        yt = yp.tile([128, 512], f32, name=f"yt{s}_{kd}_{int(c0)}", tag="yt")
                        if last_group and i == len(tiles) - 1:
                            # shorten the drain: copy + store the final tile
                            # as thirds on independent engines/queues
                            t1, t2 = tw // 3, 2 * tw // 3
                            nc.vector.tensor_copy(yt[:, :t1], p[:, :t1])
                            nc.sync.dma_start(y_d[s][kd, :, c0 : c0 + t1], yt[:, :t1])
                            nc.vector.tensor_copy(yt[:, t1:t2], p[:, t1:t2])
                            nc.gpsimd.dma_start(
                                y_d[s][kd, :, c0 + t1 : c0 + t2], yt[:, t1:t2]
                            )
                            nc.scalar.copy(yt[:, t2:tw], p[:, t2:tw])
                            nc.scalar.dma_start(
                                y_d[s][kd, :, c0 + t2 : c0 + tw], yt[:, t2:tw]
                            )
                            continue
                        nc.vector.tensor_copy(yt[:, :tw], p[:, :tw])
                        yeng = nc.scalar if (kd + i) % 2 == 0 else nc.sync
                        yeng.dma_start(y_d[s][kd, :, c0 : c0 + tw], yt[:, :tw])

    nc.compile()
    return nc


def _route(x2d, Wg, k):
    logits = x2d.astype(np.float32) @ Wg.astype(np.float32)  # [T, E]
    idx = np.argsort(-logits, axis=1, kind="stable")[:, :k]  # [T, k]
    vals = np.take_along_axis(logits, idx, axis=1)
    e = np.exp(vals - vals.max(axis=1, keepdims=True))
    w = (e / e.sum(axis=1, keepdims=True)).astype(np.float32)
    return idx, w


def kernel(x, W1, b1, W2, b2, Wg, k):
    import ml_dtypes
    from concourse.bass_utils import run_bass_kernel_spmd

    bf16 = ml_dtypes.bfloat16

    x = np.asarray(x, np.float32)
    W1 = np.asarray(W1, np.float32)
    b1 = np.asarray(b1, np.float32)
    W2 = np.asarray(W2, np.float32)
    b2 = np.asarray(b2, np.float32)
    Wg = np.asarray(Wg, np.float32)
    k = int(k)

    B, T, D = x.shape
    x2d = np.ascontiguousarray(x.reshape(-1, D))
    n_tok = x2d.shape[0]

    idx, w = _route(x2d, Wg, k)

    # bucket tokens per expert
    tok_lists, wt_lists = [], []
    for e in range(N_EXPERTS):
        sel = np.nonzero(idx == e)
        tok_lists.append(sel[0].astype(np.int64))
        wt_lists.append(w[sel[0], sel[1]])

    counts = np.array([len(t) for t in tok_lists])
    order = np.argsort(-counts, kind="stable")  # experts sorted by count desc
    big, small = order[:N_CORES], order[N_CORES:]

    def _pad(c):
        return max(128, ((int(c) + 1) // 2) * 2)

    C0 = _pad(counts[big].max())
    C1 = _pad(counts[small].max())

    key = (C0, C1)
    nc = _PROG_CACHE.get(key)
    if nc is None:
        nc = _build_program(C0, C1)
        _PROG_CACHE[key] = nc

    # host-side weight relayout (bf16, matmul-native block layout)
    w1_host = np.ascontiguousarray(
        W1.reshape(N_EXPERTS, KD, 128, KF, 128).transpose(0, 3, 2, 1, 4)
    ).astype(bf16)
    w2_host = np.ascontiguousarray(
        W2.reshape(N_EXPERTS, KF, 128, KD, 128).transpose(0, 3, 2, 1, 4)
    ).astype(bf16)
    b1_host = np.ascontiguousarray(b1.reshape(N_EXPERTS, KF, 128).transpose(0, 2, 1))
    x_bf = x2d.astype(bf16)

    in_maps = []
    for c in range(N_CORES):
        m = {}
        for s, (experts, C) in enumerate(((big, C0), (small, C1))):
            e = int(experts[c])
            toks = tok_lists[e]
            cnt = len(toks)
            xt = np.zeros((128, KD, C), bf16)
            # xt[p, kd, c] = x[token c, kd*128 + p]
            xt[:, :, :cnt] = x_bf[toks].reshape(cnt, KD, 128).transpose(2, 1, 0)
            m[f"xt{s}"] = xt
            m[f"w1_{s}"] = w1_host[e]
            m[f"w2_{s}"] = w2_host[e]
            m[f"b1_{s}"] = b1_host[e]
        in_maps.append(m)

    import os

    trace = bool(os.environ.get("MOE_TRACE"))
    r = run_bass_kernel_spmd(nc, in_maps, list(range(N_CORES)), trace=trace)
    global last_results
    last_results = r
    res = r.results

    out = np.zeros((n_tok, D_MODEL), np.float32)
    for c in range(N_CORES):
        for s, experts in enumerate((big, small)):
            e = int(experts[c])
            toks = tok_lists[e]
            cnt = len(toks)
            y = res[c][f"y{s}"]  # [KD, 128, C] = expert output, [d, token]
            contrib = y.reshape(D_MODEL, -1)[:, :cnt].T * wt_lists[e][:, None]
            if b2[e].any():
                contrib = contrib + wt_lists[e][:, None] * b2[e][None, :]
            out[toks] += contrib  # token ids unique within one expert
    return out.reshape(B, T, D_MODEL)


# revision 20
# speedup vs baseline: 1.0042x; 1.0042x over previous
"""MoE layer (16 experts, top-2) on 8 Trainium2 NeuronCores, expert-parallel.

Strategy:
  - Host computes the gating (logits -> top-k -> softmax) and routes tokens
    into per-expert buckets (the shard/dispatch step).
  - Experts are sorted by token count; the 8 largest go to slot 0 (padded to
    C0 = max count), the 8 smallest to slot 1 (padded to C1 = 9th largest
    count).  One big + one small expert per core: per-core padded work is
    C0 + C1 ~ 1080 tokens instead of 2*C0.
  - All matmuls in bf16 (full PE rate + FWL fast weight loads + half the HBM
    traffic of fp32), fp32 PSUM accumulation.
  - matmul1: ht[f, t] = silu(W1.T @ xt + b1), f on partitions, tokens moving.
  - matmul2: y[d, t]  = W2.T @ ht, d on partitions, tokens moving -- weights
    are always the stationary operand and the moving dim is the exact token
    count (no ceil(C/128) partition-padding waste).
  - Host combines: out[token] = sum over its top-k experts of
    gate * y[:, token] (the unshard/combine step; gate applied on host).
"""

import math

import numpy as np

D_MODEL = 1024
D_FF = 4096
N_EXPERTS = 16
N_CORES = 8
SLOTS = 2  # experts per core
KD = D_MODEL // 128  # 8 contraction chunks for matmul1 / output chunks for mm2
KF = D_FF // 128  # 32 f chunks

_PROG_CACHE: dict[tuple, object] = {}


def _split_tokens(c):
    """Split token count c into moving-dim tiles <= 512 (PSUM bank limit),
    as equal as possible (each >= 256 for c >= 512)."""
    n = max(1, math.ceil(c / 512))
    q, r = divmod(c, n)
    sizes = [q + (1 if i < r else 0) for i in range(n)]
    out = []
    c0 = 0
    for sz in sizes:
        out.append((c0, sz))
        c0 += sz
    return out


def _build_program(C0, C1):
    import concourse.bass as bass  # noqa: F401
    import concourse.tile as tile
    from concourse import bacc, mybir

    f32 = mybir.dt.float32
    bf16 = mybir.dt.bfloat16
    silu = mybir.ActivationFunctionType.Silu

    nc = bacc.Bacc("TRN2", target_bir_lowering=False, debug=False, num_devices=N_CORES)

    CS = [C0, C1]
    xt_d, w1_d, w2_d, b1_d, y_d = [], [], [], [], []
    for s, C in enumerate(CS):
        xt_d.append(nc.dram_tensor(f"xt{s}", [128, KD, C], bf16, kind="ExternalInput").ap())
        w1_d.append(nc.dram_tensor(f"w1_{s}", [KF, 128, KD, 128], bf16, kind="ExternalInput").ap())
        w2_d.append(nc.dram_tensor(f"w2_{s}", [KD, 128, KF, 128], bf16, kind="ExternalInput").ap())
        b1_d.append(nc.dram_tensor(f"b1_{s}", [128, KF], f32, kind="ExternalInput").ap())
        y_d.append(nc.dram_tensor(f"y{s}", [KD, 128, C], f32, kind="ExternalOutput").ap())

    with tile.TileContext(nc) as tc:
        with (
            tc.tile_pool(name="xtp", bufs=1) as xtp,
            tc.tile_pool(name="w1p", bufs=12) as w1p,
            tc.tile_pool(name="w2p", bufs=3) as w2p,
            tc.tile_pool(name="htp", bufs=1) as htp,
            tc.tile_pool(name="smallp", bufs=2) as smallp,
            tc.tile_pool(name="yp", bufs=4) as yp,
            tc.tile_pool(name="ps1", bufs=4, space="PSUM") as ps1,
            tc.tile_pool(name="ps2", bufs=4, space="PSUM") as ps2,
        ):
            # (No HAM pre-warm: the startup window is HBM-bound — a warm PE
            # just outruns the x/W1 supply and stalls; measured net-negative.)
            sorder = [0, 1]
            for oi, s in enumerate(sorder):
                C = CS[s]
                tiles = _split_tokens(C)

                # ---- loads for this expert ----
                # per-kd x chunks on two queues so the first matmul can start
                # as soon as chunk 0 + the first W1 block land
                # chunks in consumption order (kd descending), spread over all
                # three queues, ahead of the w1 stream on sync
                xt = xtp.tile([128, KD, C], bf16, name=f"xt{s}", tag="xt")
                nc.sync.dma_start(xt[:, 6:8], xt_d[s][:, 6:8])
                nc.scalar.dma_start(xt[:, 4:6], xt_d[s][:, 4:6])
                nc.gpsimd.dma_start(xt[:, 2:4], xt_d[s][:, 2:4])
                nc.scalar.dma_start(xt[:, 0:2], xt_d[s][:, 0:2])
                b1t = smallp.tile([128, KF], f32, name=f"b1t{s}", tag="b1t")
                nc.gpsimd.dma_start(b1t[:], b1_d[s])

                # ---- matmul1: ht[f, c] = silu(W1.T @ xt + b1) ----
                # w2 blocks: the first two ride the sync queue positioned
                # behind 8 w1 issues (keeps them out of the startup window);
                # the rest are WAR-gated just-in-time by the bufs=2 rotation
                # against matmul2's progress.
                ht = htp.tile([128, KF, C], bf16, name=f"ht{s}", tag="ht")
                w2ts = []
                for kf in range(KF):
                    w1t = w1p.tile([128, KD, 128], bf16, name=f"w1t{s}_{kf}", tag="w1t")
                    nc.sync.dma_start(w1t[:], w1_d[s][kf])
                    if kf in (8, 12, 14) or (16 <= kf and kf % 2 == 0 and len(w2ts) < KD):
                        kd = len(w2ts)
                        w2t = w2p.tile(
                            [128, KF, 128], bf16, name=f"w2t{s}_{kd}", tag="w2t"
                        )
                        # first three ride the scalar DMA ring (idle, and
                        # queue-ordered behind this slot's earlier silus);
                        # the rest are WAR-gated just-in-time on gpsimd
                        w2eng = nc.scalar if kd < 3 else nc.gpsimd
                        w2eng.dma_start(w2t[:], w2_d[s][kd])
                        w2ts.append(w2t)
                    pt = [
                        ps1.tile([128, 512], f32, name=f"ps1_{s}_{kf}_{i}", tag="ps1")
                        for i in range(len(tiles))
                    ]
                    # kd descending: the first matmul gates on the last x
                    # chunk, so the w1 stream banks a cushion during the
                    # x load instead of stuttering chunk-by-chunk
                    for j, kd in enumerate(reversed(range(KD))):
                        for p, (c0, tw) in zip(pt, tiles):
                            nc.tensor.matmul(
                                p[:, :tw],
                                lhsT=w1t[:, kd],
                                rhs=xt[:, kd, c0 : c0 + tw],
                                start=(j == 0),
                                stop=(j == KD - 1),
                            )
                    for p, (c0, tw) in zip(pt, tiles):
                        nc.scalar.activation(
                            ht[:, kf, c0 : c0 + tw],
                            p[:, :tw],
                            silu,
                            bias=b1t[:, kf : kf + 1],
                        )

                # ---- matmul2: y[d, c] = W2.T @ ht ----
                for kd in range(KD):
                    w2t = w2ts[kd]
                    pt2 = [
                        ps2.tile([128, 512], f32, name=f"ps2_{s}_{kd}_{i}", tag="ps2")
                        for i in range(len(tiles))
                    ]
                    for kf in range(KF):
                        for p, (c0, tw) in zip(pt2, tiles):
                            nc.tensor.matmul(
                                p[:, :tw],
                                lhsT=w2t[:, kf],
                                rhs=ht[:, kf, c0 : c0 + tw],
                                start=(kf == 0),
                                stop=(kf == KF - 1),
                            )
                    last_group = oi == len(sorder) - 1 and kd == KD - 1
                    for i, (p, (c0, tw)) in enumerate(zip(pt2, tiles)):
                        yt = yp.tile([128, 512], f32, name=f"yt{s}_{kd}_{int(c0)}", tag="yt")
                        if last_group and i == len(tiles) - 1:
                            # shorten the drain: copy + store the final tile
                            # as thirds on independent engines/queues
                            t1, t2 = tw // 3, 2 * tw // 3
                            nc.vector.tensor_copy(yt[:, :t1], p[:, :t1])
                            nc.sync.dma_start(y_d[s][kd, :, c0 : c0 + t1], yt[:, :t1])
                            nc.vector.tensor_copy(yt[:, t1:t2], p[:, t1:t2])
                            nc.gpsimd.dma_start(
                                y_d[s][kd, :, c0 + t1 : c0 + t2], yt[:, t1:t2]
                            )
                            nc.scalar.copy(yt[:, t2:tw], p[:, t2:tw])
                            nc.scalar.dma_start(
                                y_d[s][kd, :, c0 + t2 : c0 + tw], yt[:, t2:tw]
                            )
                            continue
                        nc.vector.tensor_copy(yt[:, :tw], p[:, :tw])
                        yeng = nc.scalar if (kd + i) % 2 == 0 else nc.sync
                        yeng.dma_start(y_d[s][kd, :, c0 : c0 + tw], yt[:, :tw])

    nc.compile()
    return nc


def _route(x2d, Wg, k):
    logits = x2d.astype(np.float32) @ Wg.astype(np.float32)  # [T, E]
    idx = np.argsort(-logits, axis=1, kind="stable")[:, :k]  # [T, k]
    vals = np.take_along_axis(logits, idx, axis=1)
    e = np.exp(vals - vals.max(axis=1, keepdims=True))
    w = (e / e.sum(axis=1, keepdims=True)).astype(np.float32)
    return idx, w


def kernel(x, W1, b1, W2, b2, Wg, k):
    import ml_dtypes
    from concourse.bass_utils import run_bass_kernel_spmd

    bf16 = ml_dtypes.bfloat16

    x = np.asarray(x, np.float32)
    W1 = np.asarray(W1, np.float32)
    b1 = np.asarray(b1, np.float32)
    W2 = np.asarray(W2, np.float32)
    b2 = np.asarray(b2, np.float32)
    Wg = np.asarray(Wg, np.float32)
    k = int(k)

    B, T, D = x.shape
    x2d = np.ascontiguousarray(x.reshape(-1, D))
    n_tok = x2d.shape[0]

    idx, w = _route(x2d, Wg, k)

    # bucket tokens per expert
    tok_lists, wt_lists = [], []
    for e in range(N_EXPERTS):
        sel = np.nonzero(idx == e)
        tok_lists.append(sel[0].astype(np.int64))
        wt_lists.append(w[sel[0], sel[1]])

    counts = np.array([len(t) for t in tok_lists])
    order = np.argsort(-counts, kind="stable")  # experts sorted by count desc
    big, small = order[:N_CORES], order[N_CORES:]

    def _pad(c):
        return max(128, ((int(c) + 1) // 2) * 2)

    C0 = _pad(counts[big].max())
    C1 = _pad(counts[small].max())

    key = (C0, C1)
    nc = _PROG_CACHE.get(key)
    if nc is None:
        nc = _build_program(C0, C1)
        _PROG_CACHE[key] = nc

    # host-side weight relayout (bf16, matmul-native block layout)
    w1_host = np.ascontiguousarray(
        W1.reshape(N_EXPERTS, KD, 128, KF, 128).transpose(0, 3, 2, 1, 4)
    ).astype(bf16)
    w2_host = np.ascontiguousarray(
        W2.reshape(N_EXPERTS, KF, 128, KD, 128).transpose(0, 3, 2, 1, 4)
    ).astype(bf16)
    b1_host = np.ascontiguousarray(b1.reshape(N_EXPERTS, KF, 128).transpose(0, 2, 1))
    x_bf = x2d.astype(bf16)

    in_maps = []
    for c in range(N_CORES):
        m = {}
        for s, (experts, C) in enumerate(((big, C0), (small, C1))):
            e = int(experts[c])
            toks = tok_lists[e]
            cnt = len(toks)
            xt = np.zeros((128, KD, C), bf16)
            # xt[p, kd, c] = x[token c, kd*128 + p]
            xt[:, :, :cnt] = x_bf[toks].reshape(cnt, KD, 128).transpose(2, 1, 0)
            m[f"xt{s}"] = xt
            m[f"w1_{s}"] = w1_host[e]
            m[f"w2_{s}"] = w2_host[e]
            m[f"b1_{s}"] = b1_host[e]
        in_maps.append(m)

    import os

    trace = bool(os.environ.get("MOE_TRACE"))
    r = run_bass_kernel_spmd(nc, in_maps, list(range(N_CORES)), trace=trace)
    global last_results
    last_results = r
    res = r.results

    out = np.zeros((n_tok, D_MODEL), np.float32)
    for c in range(N_CORES):
        for s, experts in enumerate((big, small)):
            e = int(experts[c])
            toks = tok_lists[e]
            cnt = len(toks)
            y = res[c][f"y{s}"]  # [KD, 128, C] = expert output, [d, token]
            contrib = y.reshape(D_MODEL, -1)[:, :cnt].T * wt_lists[e][:, None]
            if b2[e].any():
                contrib = contrib + wt_lists[e][:, None] * b2[e][None, :]
            out[toks] += contrib  # token ids unique within one expert
    return out.reshape(B, T, D_MODEL)


# revision 22
# speedup vs baseline: 1.0104x; 1.0062x over previous
"""MoE layer (16 experts, top-2) on 8 Trainium2 NeuronCores, expert-parallel.

Strategy:
  - Host computes the gating (logits -> top-k -> softmax) and routes tokens
    into per-expert buckets (the shard/dispatch step).
  - Experts are sorted by token count; the 8 largest go to slot 0 (padded to
    C0 = max count), the 8 smallest to slot 1 (padded to C1 = 9th largest
    count).  One big + one small expert per core: per-core padded work is
    C0 + C1 ~ 1080 tokens instead of 2*C0.
  - All matmuls in bf16 (full PE rate + FWL fast weight loads + half the HBM
    traffic of fp32), fp32 PSUM accumulation.
  - matmul1: ht[f, t] = silu(W1.T @ xt + b1), f on partitions, tokens moving.
  - matmul2: y[d, t]  = W2.T @ ht, d on partitions, tokens moving -- weights
    are always the stationary operand and the moving dim is the exact token
    count (no ceil(C/128) partition-padding waste).
  - Host combines: out[token] = sum over its top-k experts of
    gate * y[:, token] (the unshard/combine step; gate applied on host).
"""

import math

import numpy as np

D_MODEL = 1024
D_FF = 4096
N_EXPERTS = 16
N_CORES = 8
SLOTS = 2  # experts per core
KD = D_MODEL // 128  # 8 contraction chunks for matmul1 / output chunks for mm2
KF = D_FF // 128  # 32 f chunks

_PROG_CACHE: dict[tuple, object] = {}


def _split_tokens(c):
    """Split token count c into moving-dim tiles <= 512 (PSUM bank limit),
    as equal as possible (each >= 256 for c >= 512)."""
    n = max(1, math.ceil(c / 512))
    q, r = divmod(c, n)
    sizes = [q + (1 if i < r else 0) for i in range(n)]
    out = []
    c0 = 0
    for sz in sizes:
        out.append((c0, sz))
        c0 += sz
    return out


def _build_program(C0, C1):
    import concourse.bass as bass  # noqa: F401
    import concourse.tile as tile
    from concourse import bacc, mybir

    f32 = mybir.dt.float32
    bf16 = mybir.dt.bfloat16
    silu = mybir.ActivationFunctionType.Silu

    nc = bacc.Bacc("TRN2", target_bir_lowering=False, debug=False, num_devices=N_CORES)

    CS = [C0, C1]
    xt_d, w1_d, w2_d, b1_d, y_d = [], [], [], [], []
    for s, C in enumerate(CS):
        xt_d.append(nc.dram_tensor(f"xt{s}", [128, KD, C], bf16, kind="ExternalInput").ap())
        w1_d.append(nc.dram_tensor(f"w1_{s}", [KF, 128, KD, 128], bf16, kind="ExternalInput").ap())
        w2_d.append(nc.dram_tensor(f"w2_{s}", [KD, 128, KF, 128], bf16, kind="ExternalInput").ap())
        b1_d.append(nc.dram_tensor(f"b1_{s}", [128, KF], f32, kind="ExternalInput").ap())
        y_d.append(nc.dram_tensor(f"y{s}", [KD, 128, C], f32, kind="ExternalOutput").ap())

    with tile.TileContext(nc) as tc:
        with (
            tc.tile_pool(name="xtp", bufs=1) as xtp,
            tc.tile_pool(name="w1p", bufs=12) as w1p,
            tc.tile_pool(name="w2p", bufs=3) as w2p,
            tc.tile_pool(name="htp", bufs=1) as htp,
            tc.tile_pool(name="smallp", bufs=2) as smallp,
            tc.tile_pool(name="yp", bufs=4) as yp,
            tc.tile_pool(name="ps1", bufs=4, space="PSUM") as ps1,
            tc.tile_pool(name="ps2", bufs=4, space="PSUM") as ps2,
        ):
            # (No HAM pre-warm: the startup window is HBM-bound — a warm PE
            # just outruns the x/W1 supply and stalls; measured net-negative.)
            sorder = [0, 1]
            for oi, s in enumerate(sorder):
                C = CS[s]
                tiles = _split_tokens(C)

                # ---- loads for this expert ----
                # per-kd x chunks on two queues so the first matmul can start
                # as soon as chunk 0 + the first W1 block land
                # chunks in consumption order (kd descending), spread over all
                # three queues, ahead of the w1 stream on sync.  For the first
                # slot, the first W1 block rides scalar in parallel with the
                # first x chunk on sync so the PE can start ~3us earlier.
                w1t0 = None
                if oi == 0:
                    w1t0 = w1p.tile([128, KD, 128], bf16, name=f"w1t{s}_0", tag="w1t")
                    nc.scalar.dma_start(w1t0[:], w1_d[s][0])
                xt = xtp.tile([128, KD, C], bf16, name=f"xt{s}", tag="xt")
                nc.sync.dma_start(xt[:, 6:8], xt_d[s][:, 6:8])
                nc.gpsimd.dma_start(xt[:, 2:4], xt_d[s][:, 2:4])
                nc.scalar.dma_start(xt[:, 4:6], xt_d[s][:, 4:6])
                nc.scalar.dma_start(xt[:, 0:2], xt_d[s][:, 0:2])
                b1t = smallp.tile([128, KF], f32, name=f"b1t{s}", tag="b1t")
                nc.gpsimd.dma_start(b1t[:], b1_d[s])

                # ---- matmul1: ht[f, c] = silu(W1.T @ xt + b1) ----
                # w2 blocks: the first two ride the sync queue positioned
                # behind 8 w1 issues (keeps them out of the startup window);
                # the rest are WAR-gated just-in-time by the bufs=2 rotation
                # against matmul2's progress.
                ht = htp.tile([128, KF, C], bf16, name=f"ht{s}", tag="ht")
                w2ts = []
                for kf in range(KF):
                    if kf == 0 and w1t0 is not None:
                        w1t = w1t0
                    else:
                        w1t = w1p.tile([128, KD, 128], bf16, name=f"w1t{s}_{kf}", tag="w1t")
                        nc.sync.dma_start(w1t[:], w1_d[s][kf])
                    if kf in (8, 12, 14) or (16 <= kf and kf % 2 == 0 and len(w2ts) < KD):
                        kd = len(w2ts)
                        w2t = w2p.tile(
                            [128, KF, 128], bf16, name=f"w2t{s}_{kd}", tag="w2t"
                        )
                        # first three ride the scalar DMA ring (idle, and
                        # queue-ordered behind this slot's earlier silus);
                        # the rest are WAR-gated just-in-time on gpsimd
                        w2eng = nc.scalar if kd < 3 else nc.gpsimd
                        w2eng.dma_start(w2t[:], w2_d[s][kd])
                        w2ts.append(w2t)
                    pt = [
                        ps1.tile([128, 512], f32, name=f"ps1_{s}_{kf}_{i}", tag="ps1")
                        for i in range(len(tiles))
                    ]
                    # kd descending: the first matmul gates on the last x
                    # chunk, so the w1 stream banks a cushion during the
                    # x load instead of stuttering chunk-by-chunk
                    for j, kd in enumerate(reversed(range(KD))):
                        for p, (c0, tw) in zip(pt, tiles):
                            nc.tensor.matmul(
                                p[:, :tw],
                                lhsT=w1t[:, kd],
                                rhs=xt[:, kd, c0 : c0 + tw],
                                start=(j == 0),
                                stop=(j == KD - 1),
                            )
                    for p, (c0, tw) in zip(pt, tiles):
                        nc.scalar.activation(
                            ht[:, kf, c0 : c0 + tw],
                            p[:, :tw],
                            silu,
                            bias=b1t[:, kf : kf + 1],
                        )

                # ---- matmul2: y[d, c] = W2.T @ ht ----
                for kd in range(KD):
                    w2t = w2ts[kd]
                    pt2 = [
                        ps2.tile([128, 512], f32, name=f"ps2_{s}_{kd}_{i}", tag="ps2")
                        for i in range(len(tiles))
                    ]
                    for kf in range(KF):
                        for p, (c0, tw) in zip(pt2, tiles):
                            nc.tensor.matmul(
                                p[:, :tw],
                                lhsT=w2t[:, kf],
                                rhs=ht[:, kf, c0 : c0 + tw],
                                start=(kf == 0),
                                stop=(kf == KF - 1),
                            )
                    last_group = oi == len(sorder) - 1 and kd == KD - 1
                    for i, (p, (c0, tw)) in enumerate(zip(pt2, tiles)):
                        yt = yp.tile([128, 512], f32, name=f"yt{s}_{kd}_{int(c0)}", tag="yt")
                        if last_group and i == len(tiles) - 1:
                            # shorten the drain: copy + store the final tile
                            # as thirds on independent engines/queues
                            t1, t2 = tw // 3, 2 * tw // 3
                            nc.vector.tensor_copy(yt[:, :t1], p[:, :t1])
                            nc.sync.dma_start(y_d[s][kd, :, c0 : c0 + t1], yt[:, :t1])
                            nc.vector.tensor_copy(yt[:, t1:t2], p[:, t1:t2])
                            nc.gpsimd.dma_start(
                                y_d[s][kd, :, c0 + t1 : c0 + t2], yt[:, t1:t2]
                            )
                            nc.scalar.copy(yt[:, t2:tw], p[:, t2:tw])
                            nc.scalar.dma_start(
                                y_d[s][kd, :, c0 + t2 : c0 + tw], yt[:, t2:tw]
                            )
                            continue
                        nc.vector.tensor_copy(yt[:, :tw], p[:, :tw])
                        yeng = nc.scalar if (kd + i) % 2 == 0 else nc.sync
                        yeng.dma_start(y_d[s][kd, :, c0 : c0 + tw], yt[:, :tw])

    nc.compile()
    return nc


def _route(x2d, Wg, k):
    logits = x2d.astype(np.float32) @ Wg.astype(np.float32)  # [T, E]
    idx = np.argsort(-logits, axis=1, kind="stable")[:, :k]  # [T, k]
    vals = np.take_along_axis(logits, idx, axis=1)
    e = np.exp(vals - vals.max(axis=1, keepdims=True))
    w = (e / e.sum(axis=1, keepdims=True)).astype(np.float32)
    return idx, w


def kernel(x, W1, b1, W2, b2, Wg, k):
    import ml_dtypes
    from concourse.bass_utils import run_bass_kernel_spmd

    bf16 = ml_dtypes.bfloat16

    x = np.asarray(x, np.float32)
    W1 = np.asarray(W1, np.float32)
    b1 = np.asarray(b1, np.float32)
    W2 = np.asarray(W2, np.float32)
    b2 = np.asarray(b2, np.float32)
    Wg = np.asarray(Wg, np.float32)
    k = int(k)

    B, T, D = x.shape
    x2d = np.ascontiguousarray(x.reshape(-1, D))
    n_tok = x2d.shape[0]

    idx, w = _route(x2d, Wg, k)

    # bucket tokens per expert
    tok_lists, wt_lists = [], []
    for e in range(N_EXPERTS):
        sel = np.nonzero(idx == e)
        tok_lists.append(sel[0].astype(np.int64))
        wt_lists.append(w[sel[0], sel[1]])

    counts = np.array([len(t) for t in tok_lists])
    order = np.argsort(-counts, kind="stable")  # experts sorted by count desc
    big, small = order[:N_CORES], order[N_CORES:]

    def _pad(c):
        return max(128, ((int(c) + 1) // 2) * 2)

    C0 = _pad(counts[big].max())
    C1 = _pad(counts[small].max())

    key = (C0, C1)
    nc = _PROG_CACHE.get(key)
    if nc is None:
        nc = _build_program(C0, C1)
        _PROG_CACHE[key] = nc

    # host-side weight relayout (bf16, matmul-native block layout)
    w1_host = np.ascontiguousarray(
        W1.reshape(N_EXPERTS, KD, 128, KF, 128).transpose(0, 3, 2, 1, 4)
    ).astype(bf16)
    w2_host = np.ascontiguousarray(
        W2.reshape(N_EXPERTS, KF, 128, KD, 128).transpose(0, 3, 2, 1, 4)
    ).astype(bf16)
    b1_host = np.ascontiguousarray(b1.reshape(N_EXPERTS, KF, 128).transpose(0, 2, 1))
    x_bf = x2d.astype(bf16)

    in_maps = []
    for c in range(N_CORES):
        m = {}
        for s, (experts, C) in enumerate(((big, C0), (small, C1))):
            e = int(experts[c])
            toks = tok_lists[e]
            cnt = len(toks)
            xt = np.zeros((128, KD, C), bf16)
            # xt[p, kd, c] = x[token c, kd*128 + p]
            xt[:, :, :cnt] = x_bf[toks].reshape(cnt, KD, 128).transpose(2, 1, 0)
            m[f"xt{s}"] = xt
            m[f"w1_{s}"] = w1_host[e]
            m[f"w2_{s}"] = w2_host[e]
            m[f"b1_{s}"] = b1_host[e]
        in_maps.append(m)

    import os

    trace = bool(os.environ.get("MOE_TRACE"))
    r = run_bass_kernel_spmd(nc, in_maps, list(range(N_CORES)), trace=trace)
    global last_results
    last_results = r
    res = r.results

    out = np.zeros((n_tok, D_MODEL), np.float32)
    for c in range(N_CORES):
        for s, experts in enumerate((big, small)):
            e = int(experts[c])
            toks = tok_lists[e]
            cnt = len(toks)
            y = res[c][f"y{s}"]  # [KD, 128, C] = expert output, [d, token]
            contrib = y.reshape(D_MODEL, -1)[:, :cnt].T * wt_lists[e][:, None]
            if b2[e].any():
                contrib = contrib + wt_lists[e][:, None] * b2[e][None, :]
            out[toks] += contrib  # token ids unique within one expert
    return out.reshape(B, T, D_MODEL)
